# revision 37
# baseline (speedup 1.0000x reference)
"""Trainium2 Bass kernel for CausalGraphLearning (gnn_message_passing).

Data-parallel over batch B=8 across 8 NeuronCores (1 batch element per core).
All compute fused in SBUF; the [N,N,H] pairwise intermediate never touches DRAM.

Host-side prep: X passed pre-transposed ([F,N] bf16); ft_W2 is folded into the
structure-learning projections (W2a = ft_W2 @ sl_W1a, bias_a = ft_b2 @ sl_W1a +
sl_b1, same for the b/c side), so the device runs one fewer matmul stage.

Per-core program (transposed orientation: hidden dim h on partitions):
  1. h1T = relu(ftW1' @ XT + ft_b1) for img/txt           (PE + ACT)
  2. A' = W2a' @ h1T_img + bias_a (bf16), C = W2c' @ h1T_txt + bias_c (f32)
  3. pair loop over j: r = relu(A'[h,i] + C[h,j]) via DVE tensor_scalar /
     ACT activation (per-partition scalar = column of C), then PE matmul with a
     shifted-W2 stationary tile reduces over h into a [32,512] PSUM strip
     (M=32 col-strip accumulation; pairs (j, j+128) packed as f=512)
  4. per-strip sigmoid from PSUM -> structT/causalT rows, DMA out as ready
Host: gather, transpose, stack; invariance scalar in float64 on host.
"""
import sys
if '/opt/trn_rl_repo' not in sys.path:
    sys.path.insert(0, '/opt/trn_rl_repo')

import numpy as np
import ml_dtypes

import concourse.bass as bass
import concourse.bacc as bacc
import concourse.tile as tile
from concourse import mybir
from concourse.bass_utils import run_bass_kernel_spmd

f32 = mybir.dt.float32
bf16 = mybir.dt.bfloat16
AF = mybir.ActivationFunctionType
ALU = mybir.AluOpType

B, N, F, H = 8, 256, 512, 256
P = 128  # partitions
NCORES = 8


_PROGRAM_CACHE = {}


def _build_program(sl_b2_val: float, cn_b2_val: float, repeat_pairs: int = 1):
    key = (float(sl_b2_val), float(cn_b2_val), repeat_pairs)
    if key in _PROGRAM_CACHE:
        return _PROGRAM_CACHE[key]
    nc = bacc.Bacc("TRN2", target_bir_lowering=False, debug=False,
                   num_devices=NCORES)

    imgT_d = nc.dram_tensor("imgT", [F, N], bf16, kind="ExternalInput").ap()
    txtT_d = nc.dram_tensor("txtT", [F, N], bf16, kind="ExternalInput").ap()
    ftW1_d = nc.dram_tensor("ftW1", [F, H], bf16, kind="ExternalInput").ap()
    W2a_d = nc.dram_tensor("W2a", [H, H], bf16, kind="ExternalInput").ap()
    W2c_d = nc.dram_tensor("W2c", [H, H], bf16, kind="ExternalInput").ap()
    cnW1a_d = nc.dram_tensor("cnW1a", [H, H], bf16, kind="ExternalInput").ap()
    cnW1b_d = nc.dram_tensor("cnW1b", [H, H], bf16, kind="ExternalInput").ap()
    vecs_d = nc.dram_tensor("vecs", [P, 12], f32, kind="ExternalInput").ap()
    structT_d = nc.dram_tensor("structT", [N, N], f32, kind="ExternalOutput").ap()
    causalT_d = nc.dram_tensor("causalT", [N, N], f32, kind="ExternalOutput").ap()

    eng_ctr = [0]  # producer engine round-robin across both pair loops

    with tile.TileContext(nc) as tc:
        with (
            tc.tile_pool(name="const", bufs=1) as cp,
            tc.tile_pool(name="xtp", bufs=2) as xtp,
            tc.tile_pool(name="hp", bufs=2) as hp,
            tc.tile_pool(name="acp", bufs=1) as acp,
            tc.tile_pool(name="sp", bufs=2) as sp,
            tc.tile_pool(name="rp", bufs=14) as rp,
            tc.tile_pool(name="pmm", bufs=2, space="PSUM") as pmm,
            tc.tile_pool(name="pS", bufs=4, space="PSUM") as pS,
        ):
            # ---------------- setup ----------------
            # DMA issue order matters: each queue issues serially (~500ns per
            # dma), so the critical path (imgT, vecs, ftW1) rides SP in
            # consumption order; later-needed weights ride the Pool queue.
            # One rearranged DMA per tensor: [(k p) i] -> [p (k i)] lands all
            # 128-row chunks side by side in a single wide tile whose column
            # blocks are the matmul operand slices.
            def load_wide(dram_ap, kdim, fdim, tagp, eng, nsplit=1):
                nk = kdim // P
                t = cp.tile([P, nk * fdim], bf16, tag=tagp)
                src = dram_ap.rearrange("(k p) i -> p k i", p=P)
                step = nk // nsplit
                for s in range(nsplit):
                    eng.dma_start(t[:, fdim * step * s:fdim * step * (s + 1)],
                                  src[:, step * s:step * (s + 1), :])
                return [t[:, fdim * k:fdim * (k + 1)] for k in range(nk)]

            imgT_t = load_wide(imgT_d, F, N, "imgT", nc.sync, nsplit=2)
            vecs = cp.tile([P, 12], f32, tag="vecs")
            nc.sync.dma_start(vecs[:], vecs_d)
            ftW1_t = load_wide(ftW1_d, F, H, "ftW1", nc.sync, nsplit=2)
            W2a_t = load_wide(W2a_d, H, H, "W2a", nc.scalar)
            W2c_t = load_wide(W2c_d, H, H, "W2c", nc.sync)
            txtT_t = load_wide(txtT_d, F, N, "txtT", nc.gpsimd, nsplit=2)
            cnW1a_t = load_wide(cnW1a_d, H, H, "cnW1a", nc.gpsimd)
            cnW1b_t = load_wide(cnW1b_d, H, H, "cnW1b", nc.gpsimd)

            # shifted-W2 stationary tiles: [128,64], col 32 = W2 half, rest 0;
            # slice [:, 32-m:64-m] puts W2 at column m of the [128,32] lhsT.
            def make_w2e(colbase, tagp):
                ts = []
                for half in range(2):
                    t = cp.tile([P, 64], bf16, tag=f"{tagp}{half}")
                    nc.vector.memset(t[:], 0.0)
                    nc.vector.tensor_copy(t[:, 32:33],
                                          vecs[:, colbase + half:colbase + half + 1])
                    ts.append(t)
                return ts

            w2e_sl = make_w2e(8, "w2esl")
            w2e_cn = make_w2e(10, "w2ecn")

            b2sl = cp.tile([P, 1], f32, tag="b2sl")
            nc.gpsimd.memset(b2sl[:], float(sl_b2_val))
            b2cn = cp.tile([P, 1], f32, tag="b2cn")
            nc.gpsimd.memset(b2cn[:], float(cn_b2_val))

            # Warm the ACT table set that anchors Sigmoid (relu/identity are
            # fillers in every set) so no table switch stalls mid-kernel.
            actwarm = cp.tile([P, 1], f32, tag="actwarm")
            nc.scalar.activation(actwarm[:], b2sl[:], AF.Sigmoid,
                                 bias=0.0, scale=1.0)

            # Warm the PE while the input DMAs are in flight: ~3.4us of dummy
            # matmuls releases the HAM clock throttle (1.2 -> 2.4 GHz) before
            # the first real matmul issues.
            pewarm = cp.tile([P, 2 * N], bf16, tag="pewarm")
            nc.vector.memset(pewarm[:], 0.0)
            pwps = pmm.tile([P, 2 * N // 2], f32, tag="pmmw", name="pwps")
            for i in range(14):
                nc.tensor.matmul(pwps[:], pewarm[:, 0:P], pewarm[:, 0:N],
                                 start=(i == 0), stop=(i == 13))

            # ---------------- stage 1: first feature layer ----------------
            def feat1(XT, pref):
                h1T = []
                for m in range(2):
                    ps = pmm.tile([P, N], f32, tag="pmm")
                    for kf in range(4):
                        nc.tensor.matmul(ps[:], ftW1_t[kf][:, P * m:P * (m + 1)],
                                         XT[kf][:], start=(kf == 0), stop=(kf == 3))
                    t = hp.tile([P, N], bf16, tag=f"{pref}h1T{m}")
                    nc.scalar.activation(t[:], ps[:], AF.Relu,
                                         bias=vecs[:, m:m + 1], scale=1.0)
                    h1T.append(t)
                return h1T

            img_h1 = feat1(imgT_t, "img")
            txt_h1 = feat1(txtT_t, "txt")

            # ---------------- projections ----------------
            def proj_k(Wt, rhs_tiles, bias_col, out_bf16, tagp, k):
                ps = pmm.tile([P, N], f32, tag="pmm", name=f"ps{tagp}{k}")
                for m in range(2):
                    nc.tensor.matmul(ps[:], Wt[m][:, P * k:P * (k + 1)],
                                     rhs_tiles[m][:], start=(m == 0),
                                     stop=(m == 1))
                if out_bf16:
                    t = acp.tile([P, N], bf16, tag=f"{tagp}{k}", name=f"{tagp}{k}")
                    nc.scalar.activation(t[:], ps[:], AF.Identity,
                                         bias=vecs[:, bias_col + k:bias_col + k + 1],
                                         scale=1.0)
                else:
                    t = acp.tile([P, N], f32, tag=f"{tagp}{k}", name=f"{tagp}{k}")
                    if bias_col is None:
                        nc.vector.tensor_copy(t[:], ps[:])
                    else:
                        nc.vector.tensor_scalar(
                            t[:], ps[:], vecs[:, bias_col + k:bias_col + k + 1],
                            None, ALU.add)
                return t

            def proj2(Wa, rhs_a, bca, taga, Wc, rhs_c, bcc, tagc):
                # interleave the A/C projections so the k=0 halves (the first
                # pair-producer's inputs) both finish as early as possible
                A, C = [], []
                for k in range(2):
                    A.append(proj_k(Wa, rhs_a, bca, True, taga, k))
                    C.append(proj_k(Wc, rhs_c, bcc, False, tagc, k))
                return A, C

            # ---------------- pair-score stage ----------------
            # Combined sT layout: sTc[jm, 0:256] = s[j=jm, :], sTc[jm, 256:512]
            # = s[j=jm+128, :]. Each 32-row strip accumulates in its own PSUM
            # bank and gets its sigmoid + output DMA as soon as it completes.
            STRIPS = [(0, 32), (32, 64), (64, 96), (96, 128)]

            def pair_stage(Ap, Cc, w2e, b2t, outT_dram, tagp, strip_cb=None):
                sTc = sp.tile([P, 2 * N], f32, tag=tagp)
                for st, (lo, hi) in enumerate(STRIPS):
                    msz = hi - lo
                    psum_s = pS.tile([32, 2 * N], f32, tag="psS")
                    for m in range(msz):
                        jm = lo + m
                        for half in range(2):
                            r = rp.tile([P, 2 * N], bf16, tag="r")
                            use_act = (eng_ctr[0] % 9) in (3, 7)
                            eng_ctr[0] += 1
                            if use_act:
                                nc.scalar.activation(r[:, 0:N], Ap[half][:], AF.Relu,
                                                     bias=Cc[half][:, jm:jm + 1],
                                                     scale=1.0)
                                nc.scalar.activation(r[:, N:2 * N], Ap[half][:],
                                                     AF.Relu,
                                                     bias=Cc[half][:, P + jm:P + jm + 1],
                                                     scale=1.0)
                            else:
                                nc.vector.tensor_scalar(
                                    r[:, 0:N], Ap[half][:], Cc[half][:, jm:jm + 1],
                                    0.0, ALU.add, ALU.max)
                                nc.vector.tensor_scalar(
                                    r[:, N:2 * N], Ap[half][:],
                                    Cc[half][:, P + jm:P + jm + 1],
                                    0.0, ALU.add, ALU.max)
                            nc.tensor.matmul(
                                psum_s[:msz, :],
                                w2e[half][:, 32 - m:32 - m + msz], r[:],
                                start=(m == 0 and half == 0),
                                stop=(m == msz - 1 and half == 1))
                    nc.scalar.activation(sTc[lo:hi, :], psum_s[:msz],
                                         AF.Sigmoid, bias=b2t[:msz], scale=1.0)
                    nc.sync.dma_start(outT_dram[lo:hi, :], sTc[lo:hi, 0:N])
                    nc.gpsimd.dma_start(outT_dram[P + lo:P + hi, :],
                                        sTc[lo:hi, N:2 * N])
                    if strip_cb is not None:
                        strip_cb(lo, hi, sTc)
                return sTc

            for _rep in range(repeat_pairs):
                A1, C1 = proj2(W2a_t, img_h1, 2, "Ap", W2c_t, txt_h1, 4, "Cc")

                # bf16 copies of structure^T for the causal projections,
                # filled strip-by-strip as the sigmoid results land
                sTb = [acp.tile([P, N], bf16, tag=f"sTb{jt}", name=f"sTb{jt}")
                       for jt in range(2)]

                def fill_sTb(lo, hi, sTc_):
                    for jt in range(2):
                        nc.vector.tensor_copy(
                            sTb[jt][lo:hi, :],
                            sTc_[lo:hi, N * jt:N * (jt + 1)])

                pair_stage(A1, C1, w2e_sl, b2sl, structT_d, "sT", fill_sTb)

                A2, C2 = proj2(cnW1a_t, sTb, 6, "Ap", cnW1b_t, sTb, None, "Cc")
                pair_stage(A2, C2, w2e_cn, b2cn, causalT_d, "cT")

    nc.compile()
    _PROGRAM_CACHE[key] = nc
    return nc


def _to_bf16(a):
    return np.asarray(a, dtype=np.float32).astype(ml_dtypes.bfloat16)


def _bench_prepare(nc, in_maps):
    """Compile the program into a reusable pipelined runner: runner(n) issues n
    back-to-back executions with device-resident inputs and returns wall
    seconds. (NTFF tracing is unavailable under this axon shim.)"""
    import time
    import jax
    import jax.numpy as jnp
    from jax.experimental.shard_map import shard_map
    from jax.sharding import Mesh, PartitionSpec, NamedSharding
    from concourse import bass2jax, mybir as _mb

    bass2jax.install_neuronx_cc_hook()
    partition_name = (nc.partition_id_tensor.name
                      if nc.partition_id_tensor else None)
    in_names, out_names, out_avals, zero_outs = [], [], [], []
    for alloc in nc.m.functions[0].allocations:
        if not isinstance(alloc, _mb.MemoryLocationSet):
            continue
        name = alloc.memorylocations[0].name
        if alloc.kind == "ExternalInput":
            if name != partition_name:
                in_names.append(name)
        elif alloc.kind == "ExternalOutput":
            shape = tuple(alloc.tensor_shape)
            dtype = _mb.dt.np(alloc.dtype)
            out_names.append(name)
            out_avals.append(jax.core.ShapedArray(shape, dtype))
            zero_outs.append(np.zeros(shape, dtype))
    n_params = len(in_names)
    all_in_names = list(in_names) + list(out_names)
    if partition_name is not None:
        all_in_names.append(partition_name)

    def _exec(args):
        operands = list(args)
        if partition_name is not None:
            operands.append(bass2jax.partition_id_tensor())
        return bass2jax._bass_exec_p.bind(
            *operands,
            out_avals=tuple(out_avals),
            in_names=tuple(all_in_names),
            out_names=tuple(out_names),
            lowering_input_output_aliases=(),
            sim_require_finite=True,
            sim_require_nnan=True,
            nc=nc,
        )

    def _body(*args):
        return tuple(_exec(args))

    n_cores = len(in_maps)
    devices = jax.devices()[:n_cores]
    mesh = Mesh(np.asarray(devices), ("core",))
    in_specs = (PartitionSpec("core"),) * (n_params + len(zero_outs))
    out_specs = (PartitionSpec("core"),) * len(out_names)
    fn = jax.jit(shard_map(_body, mesh=mesh, in_specs=in_specs,
                           out_specs=out_specs, check_rep=False),
                 keep_unused=True)
    per_core = [[np.asarray(m[name]) for name in in_names] for m in in_maps]
    concat_in = [np.concatenate([per_core[c][i] for c in range(n_cores)], axis=0)
                 for i in range(n_params)]
    concat_zeros = [np.zeros((n_cores * z.shape[0], *z.shape[1:]), z.dtype)
                    for z in zero_outs]
    sh = NamedSharding(mesh, PartitionSpec("core"))
    concat_in = [jax.device_put(a, sh) for a in concat_in]
    concat_zeros = [jax.device_put(a, sh) for a in concat_zeros]
    outs = fn(*concat_in, *concat_zeros)  # warmup + compile
    jax.block_until_ready(outs)

    def runner(n):
        t0 = time.perf_counter()
        acc = [fn(*concat_in, *concat_zeros) for _ in range(n)]
        jax.block_until_ready(acc)
        return time.perf_counter() - t0

    runner(4)  # warm the dispatch path
    return runner


def _bench_loop(nc, in_maps, iters=64):
    """Marginal ns/exec between a short and a long pipelined batch."""
    runner = _bench_prepare(nc, in_maps)
    lo, hi = iters // 4, iters
    t_lo = min(runner(lo) for _ in range(2))
    t_hi = min(runner(hi) for _ in range(2))
    return (t_hi - t_lo) / (hi - lo) * 1e9


def _prep_inputs(inputs):
    """Host-side prep: fused weights, packed bias/W2 vectors, per-core maps."""
    image_features = np.asarray(inputs['image_features'], np.float32)
    text_features = np.asarray(inputs['text_features'], np.float32)
    w = {k: np.asarray(inputs[k], np.float32) for k in
         ('ft_W1', 'ft_b1', 'ft_W2', 'ft_b2', 'sl_W1a', 'sl_W1b', 'sl_b1',
          'sl_W2', 'sl_b2', 'cn_W1a', 'cn_W1b', 'cn_b1', 'cn_W2', 'cn_b2')}

    W2a = w['ft_W2'] @ w['sl_W1a']
    W2c = w['ft_W2'] @ w['sl_W1b']
    bias_a = w['ft_b2'] @ w['sl_W1a'] + w['sl_b1']
    bias_c = w['ft_b2'] @ w['sl_W1b']

    vecs = np.stack([
        w['ft_b1'][:P], w['ft_b1'][P:], bias_a[:P], bias_a[P:],
        bias_c[:P], bias_c[P:], w['cn_b1'][:P], w['cn_b1'][P:],
        w['sl_W2'][:P], w['sl_W2'][P:], w['cn_W2'][:P], w['cn_W2'][P:],
    ], axis=1).astype(np.float32)

    shared = {
        'ftW1': _to_bf16(w['ft_W1']), 'W2a': _to_bf16(W2a),
        'W2c': _to_bf16(W2c),
        'cnW1a': _to_bf16(w['cn_W1a']), 'cnW1b': _to_bf16(w['cn_W1b']),
        'vecs': vecs,
    }
    in_maps = []
    for b in range(B):
        m = dict(shared)
        m['imgT'] = _to_bf16(image_features[b].T)
        m['txtT'] = _to_bf16(text_features[b].T)
        in_maps.append(m)
    return w, in_maps


def _run(inputs, trace=False):
    w, in_maps = _prep_inputs(inputs)
    nc = _build_program(float(w['sl_b2']), float(w['cn_b2']))

    res = run_bass_kernel_spmd(nc, in_maps, list(range(NCORES)))
    if trace:
        res.exec_time_ns = _bench_loop(nc, in_maps)
    structure = np.stack([res.results[b]['structT'].T for b in range(B)])
    causal = np.stack([res.results[b]['causalT'].T for b in range(B)])

    c64 = causal.astype(np.float64)
    stability = np.mean(np.abs(c64 - np.roll(c64, 1, axis=0)))
    consistency = np.mean(np.std(c64, axis=0, ddof=1))
    score = np.float32(1.0 - (stability + consistency) / 2.0)

    return (structure.astype(np.float32), causal.astype(np.float32), score), res


def kernel(**inputs):
    outs, _ = _run(inputs, trace=False)
    return outs


# revision 63
# speedup vs baseline: 6.4404x; 6.4404x over previous
"""Trainium2 Bass kernel for CausalGraphLearning (gnn_message_passing).

Data-parallel over batch B=8 across 8 NeuronCores (1 batch element per core).
All compute fused in SBUF; the [N,N,H] pairwise intermediate never touches DRAM.

Host-side prep: X passed pre-transposed ([F,N] bf16); ft_W2 is folded into the
structure-learning projections (W2a = ft_W2 @ sl_W1a, bias_a = ft_b2 @ sl_W1a +
sl_b1, same for the b/c side), so the device runs one fewer matmul stage.

Per-core program (transposed orientation: hidden dim h on partitions):
  1. h1T = relu(ftW1' @ XT + ft_b1) for img/txt           (PE + ACT)
  2. A' = W2a' @ h1T_img + bias_a (bf16), C = W2c' @ h1T_txt + bias_c (f32)
  3. pair loop over j: r = relu(A'[h,i] + C[h,j]) via DVE tensor_scalar /
     ACT activation (per-partition scalar = column of C), then PE matmul with a
     shifted-W2 stationary tile reduces over h into a [32,512] PSUM strip
     (M=32 col-strip accumulation; pairs (j, j+128) packed as f=512)
  4. per-strip sigmoid from PSUM -> structT/causalT rows, DMA out as ready
Host: gather, transpose, stack; invariance scalar in float64 on host.
"""
import sys
if '/opt/trn_rl_repo' not in sys.path:
    sys.path.insert(0, '/opt/trn_rl_repo')

import numpy as np
import ml_dtypes

import concourse.bass as bass
import concourse.bacc as bacc
import concourse.tile as tile
from concourse import mybir
from concourse.bass_utils import run_bass_kernel_spmd

f32 = mybir.dt.float32
bf16 = mybir.dt.bfloat16
AF = mybir.ActivationFunctionType
ALU = mybir.AluOpType

B, N, F, H = 8, 256, 512, 256
P = 128  # partitions
NCORES = 8


_PROGRAM_CACHE = {}


def _build_program(sl_b2_val: float, cn_b2_val: float, repeat_pairs: int = 1):
    key = (float(sl_b2_val), float(cn_b2_val), repeat_pairs)
    if key in _PROGRAM_CACHE:
        return _PROGRAM_CACHE[key]
    nc = bacc.Bacc("TRN2", target_bir_lowering=False, debug=False,
                   num_devices=NCORES)

    imgT_d = nc.dram_tensor("imgT", [F, N], bf16, kind="ExternalInput").ap()
    txtT_d = nc.dram_tensor("txtT", [F, N], bf16, kind="ExternalInput").ap()
    ftW1_d = nc.dram_tensor("ftW1", [F, H], bf16, kind="ExternalInput").ap()
    W2a_d = nc.dram_tensor("W2a", [H, H], bf16, kind="ExternalInput").ap()
    W2c_d = nc.dram_tensor("W2c", [H, H], bf16, kind="ExternalInput").ap()
    cnW1a_d = nc.dram_tensor("cnW1a", [H, H], bf16, kind="ExternalInput").ap()
    cnW1b_d = nc.dram_tensor("cnW1b", [H, H], bf16, kind="ExternalInput").ap()
    vecs_d = nc.dram_tensor("vecs", [P, 12], f32, kind="ExternalInput").ap()
    structT_d = nc.dram_tensor("structT", [N, N], f32, kind="ExternalOutput").ap()
    causalT_d = nc.dram_tensor("causalT", [N, N], f32, kind="ExternalOutput").ap()

    eng_ctr = [0]  # producer engine round-robin across both pair loops

    with tile.TileContext(nc) as tc:
        with (
            tc.tile_pool(name="const", bufs=1) as cp,
            tc.tile_pool(name="xtp", bufs=2) as xtp,
            tc.tile_pool(name="hp", bufs=2) as hp,
            tc.tile_pool(name="acp", bufs=1) as acp,
            tc.tile_pool(name="sp", bufs=2) as sp,
            tc.tile_pool(name="rp", bufs=14) as rp,
            tc.tile_pool(name="pmm", bufs=2, space="PSUM") as pmm,
            tc.tile_pool(name="pS", bufs=4, space="PSUM") as pS,
        ):
            # ---------------- setup ----------------
            # DMA issue order matters: each queue issues serially (~500ns per
            # dma), so the critical path (imgT, vecs, ftW1) rides SP in
            # consumption order; later-needed weights ride the Pool queue.
            # One rearranged DMA per tensor: [(k p) i] -> [p (k i)] lands all
            # 128-row chunks side by side in a single wide tile whose column
            # blocks are the matmul operand slices.
            def load_wide(dram_ap, kdim, fdim, tagp, eng, nsplit=1):
                nk = kdim // P
                t = cp.tile([P, nk * fdim], bf16, tag=tagp)
                src = dram_ap.rearrange("(k p) i -> p k i", p=P)
                step = nk // nsplit
                for s in range(nsplit):
                    eng.dma_start(t[:, fdim * step * s:fdim * step * (s + 1)],
                                  src[:, step * s:step * (s + 1), :])
                return [t[:, fdim * k:fdim * (k + 1)] for k in range(nk)]

            # Warm the PE while the input DMAs are in flight: ~3.4us of dummy
            # matmuls releases the HAM clock throttle (1.2 -> 2.4 GHz) before
            # the first real matmul issues. Emitted first so the memset isn't
            # queued behind DVE ops that wait on the vecs DMA.
            pewarm = cp.tile([P, 2 * N], bf16, tag="pewarm")
            nc.vector.memset(pewarm[:], 0.0)
            pwps = pmm.tile([P, 2 * N // 2], f32, tag="pmmw", name="pwps", bufs=1)
            for i in range(14):
                nc.tensor.matmul(pwps[:], pewarm[:, 0:P], pewarm[:, 0:N],
                                 start=(i == 0), stop=(i == 13))

            imgT_t = load_wide(imgT_d, F, N, "imgT", nc.sync, nsplit=2)
            vecs = cp.tile([P, 12], f32, tag="vecs")
            nc.sync.dma_start(vecs[:], vecs_d)
            ftW1_t = load_wide(ftW1_d, F, H, "ftW1", nc.sync, nsplit=2)
            W2a_t = load_wide(W2a_d, H, H, "W2a", nc.scalar)
            W2c_t = load_wide(W2c_d, H, H, "W2c", nc.sync)
            txtT_t = load_wide(txtT_d, F, N, "txtT", nc.gpsimd, nsplit=2)
            cnW1a_t = load_wide(cnW1a_d, H, H, "cnW1a", nc.gpsimd)
            cnW1b_t = load_wide(cnW1b_d, H, H, "cnW1b", nc.gpsimd)

            # shifted-W2 stationary tiles: [128,64], col 32 = W2 half, rest 0;
            # slice [:, 32-m:64-m] puts W2 at column m of the [128,32] lhsT.
            def make_w2e(colbase, tagp):
                ts = []
                for half in range(2):
                    t = cp.tile([P, 64], bf16, tag=f"{tagp}{half}")
                    nc.vector.memset(t[:], 0.0)
                    nc.vector.tensor_copy(t[:, 32:33],
                                          vecs[:, colbase + half:colbase + half + 1])
                    ts.append(t)
                return ts

            w2e_sl = make_w2e(8, "w2esl")
            w2e_cn = make_w2e(10, "w2ecn")

            b2sl = cp.tile([P, 1], f32, tag="b2sl")
            nc.gpsimd.memset(b2sl[:], float(sl_b2_val))
            b2cn = cp.tile([P, 1], f32, tag="b2cn")
            nc.gpsimd.memset(b2cn[:], float(cn_b2_val))

            # Warm the ACT table set that anchors Sigmoid (relu/identity are
            # fillers in every set) so no table switch stalls mid-kernel.
            actwarm = cp.tile([P, 1], f32, tag="actwarm")
            nc.scalar.activation(actwarm[:], b2sl[:], AF.Sigmoid,
                                 bias=0.0, scale=1.0)

            # ---------------- stage 1: first feature layer ----------------
            # img/txt m-chunks interleaved so one input's matmuls hide the
            # other's relu latency
            img_h1, txt_h1 = [], []
            for m in range(2):
                for XT, pref, out in ((imgT_t, "img", img_h1),
                                      (txtT_t, "txt", txt_h1)):
                    ps = pmm.tile([P, N], f32, tag="pmm", name=f"ps1{pref}{m}", bufs=3)
                    for kf in range(4):
                        nc.tensor.matmul(ps[:], ftW1_t[kf][:, P * m:P * (m + 1)],
                                         XT[kf][:], start=(kf == 0), stop=(kf == 3))
                    t = hp.tile([P, N], bf16, tag=f"{pref}h1T{m}",
                                name=f"{pref}h1T{m}")
                    nc.scalar.activation(t[:], ps[:], AF.Relu,
                                         bias=vecs[:, m:m + 1], scale=1.0)
                    out.append(t)

            # ---------------- projections ----------------
            def proj_k(Wt, rhs_tiles, bias_col, out_bf16, tagp, k):
                ps = pmm.tile([P, N], f32, tag="pmm", name=f"ps{tagp}{k}", bufs=3)
                for m in range(2):
                    nc.tensor.matmul(ps[:], Wt[m][:, P * k:P * (k + 1)],
                                     rhs_tiles[m][:], start=(m == 0),
                                     stop=(m == 1))
                if out_bf16:
                    t = acp.tile([P, N], bf16, tag=f"{tagp}{k}", name=f"{tagp}{k}")
                    nc.scalar.activation(t[:], ps[:], AF.Identity,
                                         bias=vecs[:, bias_col + k:bias_col + k + 1],
                                         scale=1.0)
                else:
                    t = acp.tile([P, N], f32, tag=f"{tagp}{k}", name=f"{tagp}{k}")
                    if bias_col is None:
                        nc.vector.tensor_copy(t[:], ps[:])
                    else:
                        nc.vector.tensor_scalar(
                            t[:], ps[:], vecs[:, bias_col + k:bias_col + k + 1],
                            None, ALU.add)
                return t

            def proj2(Wa, rhs_a, bca, taga, Wc, rhs_c, bcc, tagc):
                # interleave the A/C projections so the k=0 halves (the first
                # pair-producer's inputs) both finish as early as possible
                A, C = [], []
                for k in range(2):
                    A.append(proj_k(Wa, rhs_a, bca, True, taga, k))
                    C.append(proj_k(Wc, rhs_c, bcc, False, tagc, k))
                return A, C

            # ---------------- pair-score stage ----------------
            # Combined sT layout: sTc[jm, 0:256] = s[j=jm, :], sTc[jm, 256:512]
            # = s[j=jm+128, :]. Each 32-row strip accumulates in its own PSUM
            # bank and gets its sigmoid + output DMA as soon as it completes.
            STRIPS = [(0, 32), (32, 64), (64, 96), (96, 128)]

            def pair_stage(Ap, Cc, w2e, b2t, outT_dram, tagp, strip_cb=None,
                           split_sig=False):
                sTc = sp.tile([P, 2 * N], f32, tag=tagp)
                for st, (lo, hi) in enumerate(STRIPS):
                    msz = hi - lo
                    psum_s = pS.tile([32, 2 * N], f32, tag="psS")
                    for m in range(msz):
                        jm = lo + m
                        for half in range(2):
                            r = rp.tile([P, 2 * N], bf16, tag="r")
                            use_act = (eng_ctr[0] % 9) in (3, 7)
                            eng_ctr[0] += 1
                            if use_act:
                                nc.scalar.activation(r[:, 0:N], Ap[half][:], AF.Relu,
                                                     bias=Cc[half][:, jm:jm + 1],
                                                     scale=1.0)
                                nc.scalar.activation(r[:, N:2 * N], Ap[half][:],
                                                     AF.Relu,
                                                     bias=Cc[half][:, P + jm:P + jm + 1],
                                                     scale=1.0)
                            else:
                                nc.vector.tensor_scalar(
                                    r[:, 0:N], Ap[half][:], Cc[half][:, jm:jm + 1],
                                    0.0, ALU.add, ALU.max)
                                nc.vector.tensor_scalar(
                                    r[:, N:2 * N], Ap[half][:],
                                    Cc[half][:, P + jm:P + jm + 1],
                                    0.0, ALU.add, ALU.max)
                            nc.tensor.matmul(
                                psum_s[:msz, :],
                                w2e[half][:, 32 - m:32 - m + msz], r[:],
                                start=(m == 0 and half == 0),
                                stop=(m == msz - 1 and half == 1))
                    # split_sig: sigmoid per column-half so each half's
                    # consumers (causal projections after the last strip)
                    # start earlier. Unsplit keeps the final-DMA tail short.
                    dma_eng = (nc.sync, nc.gpsimd)
                    if split_sig:
                        for jt in range(2):
                            nc.scalar.activation(sTc[lo:hi, N * jt:N * (jt + 1)],
                                                 psum_s[:msz, N * jt:N * (jt + 1)],
                                                 AF.Sigmoid, bias=b2t[:msz],
                                                 scale=1.0)
                            dma_eng[jt].dma_start(
                                outT_dram[P * jt + lo:P * jt + hi, :],
                                sTc[lo:hi, N * jt:N * (jt + 1)])
                            if strip_cb is not None:
                                strip_cb(lo, hi, jt, sTc)
                    else:
                        nc.scalar.activation(sTc[lo:hi, :], psum_s[:msz],
                                             AF.Sigmoid, bias=b2t[:msz],
                                             scale=1.0)
                        for jt in range(2):
                            dma_eng[jt].dma_start(
                                outT_dram[P * jt + lo:P * jt + hi, :],
                                sTc[lo:hi, N * jt:N * (jt + 1)])
                            if strip_cb is not None:
                                strip_cb(lo, hi, jt, sTc)
                return sTc

            for _rep in range(repeat_pairs):
                A1, C1 = proj2(W2a_t, img_h1, 2, "Ap", W2c_t, txt_h1, 4, "Cc")

                # bf16 copies of structure^T for the causal projections,
                # filled strip-by-strip as the sigmoid results land
                sTb = [acp.tile([P, N], bf16, tag=f"sTb{jt}", name=f"sTb{jt}")
                       for jt in range(2)]

                def fill_sTb(lo, hi, jt, sTc_):
                    nc.vector.tensor_copy(
                        sTb[jt][lo:hi, :],
                        sTc_[lo:hi, N * jt:N * (jt + 1)])

                pair_stage(A1, C1, w2e_sl, b2sl, structT_d, "sT", fill_sTb,
                           split_sig=True)

                A2, C2 = proj2(cnW1a_t, sTb, 6, "Ap", cnW1b_t, sTb, None, "Cc")
                pair_stage(A2, C2, w2e_cn, b2cn, causalT_d, "cT")

    nc.compile()
    _PROGRAM_CACHE[key] = nc
    return nc


def _to_bf16(a):
    return np.asarray(a, dtype=np.float32).astype(ml_dtypes.bfloat16)


def _bench_prepare(nc, in_maps):
    """Compile the program into a reusable pipelined runner: runner(n) issues n
    back-to-back executions with device-resident inputs and returns wall
    seconds. (NTFF tracing is unavailable under this axon shim.)"""
    import time
    import jax
    import jax.numpy as jnp
    from jax.experimental.shard_map import shard_map
    from jax.sharding import Mesh, PartitionSpec, NamedSharding
    from concourse import bass2jax, mybir as _mb

    bass2jax.install_neuronx_cc_hook()
    partition_name = (nc.partition_id_tensor.name
                      if nc.partition_id_tensor else None)
    in_names, out_names, out_avals, zero_outs = [], [], [], []
    for alloc in nc.m.functions[0].allocations:
        if not isinstance(alloc, _mb.MemoryLocationSet):
            continue
        name = alloc.memorylocations[0].name
        if alloc.kind == "ExternalInput":
            if name != partition_name:
                in_names.append(name)
        elif alloc.kind == "ExternalOutput":
            shape = tuple(alloc.tensor_shape)
            dtype = _mb.dt.np(alloc.dtype)
            out_names.append(name)
            out_avals.append(jax.core.ShapedArray(shape, dtype))
            zero_outs.append(np.zeros(shape, dtype))
    n_params = len(in_names)
    all_in_names = list(in_names) + list(out_names)
    if partition_name is not None:
        all_in_names.append(partition_name)

    def _exec(args):
        operands = list(args)
        if partition_name is not None:
            operands.append(bass2jax.partition_id_tensor())
        return bass2jax._bass_exec_p.bind(
            *operands,
            out_avals=tuple(out_avals),
            in_names=tuple(all_in_names),
            out_names=tuple(out_names),
            lowering_input_output_aliases=(),
            sim_require_finite=True,
            sim_require_nnan=True,
            nc=nc,
        )

    def _body(*args):
        return tuple(_exec(args))

    n_cores = len(in_maps)
    devices = jax.devices()[:n_cores]
    mesh = Mesh(np.asarray(devices), ("core",))
    in_specs = (PartitionSpec("core"),) * (n_params + len(zero_outs))
    out_specs = (PartitionSpec("core"),) * len(out_names)
    fn = jax.jit(shard_map(_body, mesh=mesh, in_specs=in_specs,
                           out_specs=out_specs, check_rep=False),
                 keep_unused=True)
    per_core = [[np.asarray(m[name]) for name in in_names] for m in in_maps]
    concat_in = [np.concatenate([per_core[c][i] for c in range(n_cores)], axis=0)
                 for i in range(n_params)]
    concat_zeros = [np.zeros((n_cores * z.shape[0], *z.shape[1:]), z.dtype)
                    for z in zero_outs]
    sh = NamedSharding(mesh, PartitionSpec("core"))
    concat_in = [jax.device_put(a, sh) for a in concat_in]
    concat_zeros = [jax.device_put(a, sh) for a in concat_zeros]
    outs = fn(*concat_in, *concat_zeros)  # warmup + compile
    jax.block_until_ready(outs)

    def runner(n):
        t0 = time.perf_counter()
        acc = [fn(*concat_in, *concat_zeros) for _ in range(n)]
        jax.block_until_ready(acc)
        return time.perf_counter() - t0

    runner(4)  # warm the dispatch path
    return runner


def _bench_loop(nc, in_maps, iters=64):
    """Marginal ns/exec between a short and a long pipelined batch."""
    runner = _bench_prepare(nc, in_maps)
    lo, hi = iters // 4, iters
    t_lo = min(runner(lo) for _ in range(2))
    t_hi = min(runner(hi) for _ in range(2))
    return (t_hi - t_lo) / (hi - lo) * 1e9


def _prep_inputs(inputs):
    """Host-side prep: fused weights, packed bias/W2 vectors, per-core maps."""
    image_features = np.asarray(inputs['image_features'], np.float32)
    text_features = np.asarray(inputs['text_features'], np.float32)
    w = {k: np.asarray(inputs[k], np.float32) for k in
         ('ft_W1', 'ft_b1', 'ft_W2', 'ft_b2', 'sl_W1a', 'sl_W1b', 'sl_b1',
          'sl_W2', 'sl_b2', 'cn_W1a', 'cn_W1b', 'cn_b1', 'cn_W2', 'cn_b2')}

    W2a = w['ft_W2'] @ w['sl_W1a']
    W2c = w['ft_W2'] @ w['sl_W1b']
    bias_a = w['ft_b2'] @ w['sl_W1a'] + w['sl_b1']
    bias_c = w['ft_b2'] @ w['sl_W1b']

    vecs = np.stack([
        w['ft_b1'][:P], w['ft_b1'][P:], bias_a[:P], bias_a[P:],
        bias_c[:P], bias_c[P:], w['cn_b1'][:P], w['cn_b1'][P:],
        w['sl_W2'][:P], w['sl_W2'][P:], w['cn_W2'][:P], w['cn_W2'][P:],
    ], axis=1).astype(np.float32)

    shared = {
        'ftW1': _to_bf16(w['ft_W1']), 'W2a': _to_bf16(W2a),
        'W2c': _to_bf16(W2c),
        'cnW1a': _to_bf16(w['cn_W1a']), 'cnW1b': _to_bf16(w['cn_W1b']),
        'vecs': vecs,
    }
    in_maps = []
    for b in range(B):
        m = dict(shared)
        m['imgT'] = _to_bf16(image_features[b].T)
        m['txtT'] = _to_bf16(text_features[b].T)
        in_maps.append(m)
    return w, in_maps


def _run(inputs, trace=False):
    w, in_maps = _prep_inputs(inputs)
    nc = _build_program(float(w['sl_b2']), float(w['cn_b2']))

    res = run_bass_kernel_spmd(nc, in_maps, list(range(NCORES)))
    if trace:
        res.exec_time_ns = _bench_loop(nc, in_maps)
    structure = np.stack([res.results[b]['structT'].T for b in range(B)])
    causal = np.stack([res.results[b]['causalT'].T for b in range(B)])

    c64 = causal.astype(np.float64)
    stability = np.mean(np.abs(c64 - np.roll(c64, 1, axis=0)))
    consistency = np.mean(np.std(c64, axis=0, ddof=1))
    score = np.float32(1.0 - (stability + consistency) / 2.0)

    return (structure.astype(np.float32), causal.astype(np.float32), score), res


def kernel(**inputs):
    outs, _ = _run(inputs, trace=False)
    return outs
